# revision 55
# baseline (speedup 1.0000x reference)
"""Trainium2 Bass kernel for nn_CNNBackbone: conv1d(D->C,K=5) + BN + ReLU,
conv1d(C->C,K=5) + BN + ReLU, conv1d(C->D,1x1), masked mean over ragged lengths.

Strategy (fp8 DoubleRow; ~56.9us vs 89.6us bf16 baseline)
---------------------------------------------------------
Data-parallel over batch: 32 samples -> 8 cores x 4 sample-slots, sorted by
length so each slot's group of 8 has near-uniform length; per-slot loop bounds
are compile-time constants from the group max (SPMD: one program, 8 cores).

Numerics: both convs run in fp8-e4m3 with MatmulPerfMode.DoubleRow (2 fp8
contraction elements per PE cell per cycle; measured MM issue gap 216ns at
N=512 = the fill-rate floor, LDWEIGHTS fully hidden).
 - conv2 pairs the two 128-channel input blocks: h1 is stored [P, 2, T] fp8,
   which is exactly the DoubleRow rhs layout (pair stride 16B-aligned).
 - conv1 pairs adjacent taps; x is DMA'd twice into a [P, 2, W] tile with a
   one-column shift so tap pairs (0,1) and (2,3) are DoubleRow matmuls and
   tap 4 is a normal fp8 matmul.
 - weights are scaled by 16 (keeps e4m3 out of subnormals); the scale is
   folded downstream (h1 stored as 16*h1, rowsums folded via host-side
   1/(16*L1), 1/(256*len) constants), so conv epilogues are a SINGLE fused
   scalar_tensor_tensor op: out = max(psum + 16*b, 0) with accum_out rowsum.
 - fp8 W2 quantization error is weight-correlated and survives pooling
   (~1.9e-2 rel); corrected by pooled += 0.5 * dW2eff @ mean(h1), folded into
   the final matvec with host-precomputed Wc = 0.5*Wf@dW2eff. mean(h1) is the
   UNMASKED mean over the computed region (a statistical estimate — masking
   it adds nothing), so it rides the conv1 epilogue accumulator for free.
   Net rel err ~8.5e-3 (numpy+CoreSim validated).

Engine split: PE matmuls; conv1 epilogue on DVE (fused stt + accum), conv2
epilogue on ScalarE ACT (bias+relu+accum); partially-masked chunks add one
fused mask-multiply-accumulate stt on DVE. The final 1x1 conv + correction
commute with the masked mean into a single bf16 matvec emitted TRANSPOSED
(rs stationary, W moving -> out [NSLOTS, P]) so the output DMA needs 4
descriptors instead of 128 (~2us less completion latency on the tail).

Schedule: ~7us fixed engine preamble; every DRAM->SBUF DMA costs ~5us of
descriptor latency, so one critical load leads each DMA ring (x-piece0-lane0
on sync, w1 then lane1 on gpsimd, biases on scalar behind the ACT table) and
14 dummy matmuls keep the PE busy until data lands (keeps the HAM clock gate
at 8/8 = 2.4GHz; any >1us PE idle hole here measurably re-throttles to
1.2GHz). Slot j+1's x loads issue before slot j's tail pieces are consumed;
per-slot chunk bounds are exact (GR=2), sliced into 512-col PSUM chunks.
"""

import math

import numpy as np
import ml_dtypes

import concourse.bass as bass
import concourse.mybir as mybir
import concourse.tile as tile
from concourse import bacc
from concourse.bass_utils import run_bass_kernel_spmd

B, S, D, C, KW = 32, 2048, 128, 256, 5
P = 128
CH = 512            # full chunk (matmul free dim / PSUM bank)
GR = 2              # tail-chunk width granularity (exact bounds, even)
NCORES = 8
NSLOTS = B // NCORES
CB = C // P         # channel blocks of 128
EPS = 1e-5
WS = 16.0           # fp8 weight scale (power of 2)
H0W = S + 16        # x pair-buffer lane width (16-aligned)
H1W = S + 16        # h1 lane width (16-aligned)
NCH = S // CH + 1   # max chunks per slot
BF16 = ml_dtypes.bfloat16
NP8 = ml_dtypes.float8_e4m3
F32 = mybir.dt.float32
BF = mybir.dt.bfloat16
F8 = mybir.dt.float8e4
DR = mybir.MatmulPerfMode.DoubleRow

_BUILD_CACHE: dict = {}
LAST_RESULTS = None  # BassKernelResults of the most recent run (for test harness)
TRACE = False        # set True (or env BASS_TRACE=1) to capture a profile


def _chunks(total):
    """Split `total` columns into 512-wide chunks plus a short tail."""
    ws = [CH] * (total // CH)
    if total % CH:
        ws.append(total % CH)
    return ws


def _build(slot_cfg):
    """Build + compile the SPMD Bass program.

    slot_cfg[j] = (L1, L2, c0): conv1/conv2 computed column counts (multiples
    of GR) and the count of fully-unmasked 512-chunks for slot j's group.
    """
    nc = bacc.Bacc(None, target_bir_lowering=False, debug=False)

    xT = nc.dram_tensor("xT", [NSLOTS, P, S], F8, kind="ExternalInput")
    msk = nc.dram_tensor("msk", [NSLOTS, S], BF, kind="ExternalInput")
    w1t = nc.dram_tensor("w1t", [P, 2, 2, CB, P], F8, kind="ExternalInput")
    w14t = nc.dram_tensor("w14t", [P, CB, P], F8, kind="ExternalInput")
    w2t = nc.dram_tensor("w2t", [P, KW, CB, 2, P], F8, kind="ExternalInput")
    wft = nc.dram_tensor("wft", [P, CB, P], BF, kind="ExternalInput")
    wct = nc.dram_tensor("wct", [P, CB, P], BF, kind="ExternalInput")
    bias1 = nc.dram_tensor("bias1", [P, CB], F32, kind="ExternalInput")
    bias2 = nc.dram_tensor("bias2", [P, CB], F32, kind="ExternalInput")
    invl1 = nc.dram_tensor("invl1", [P, NSLOTS], F32, kind="ExternalInput")
    invl2 = nc.dram_tensor("invl2", [P, NSLOTS], F32, kind="ExternalInput")
    bfr = nc.dram_tensor("bfr", [P, P], BF, kind="ExternalInput")
    indb = nc.dram_tensor("indb", [P, NSLOTS], BF, kind="ExternalInput")
    out = nc.dram_tensor("out", [NSLOTS, P], F32, kind="ExternalOutput")

    RELU = mybir.ActivationFunctionType.Relu
    ADD = mybir.AluOpType.add
    MUL = mybir.AluOpType.mult
    MAX = mybir.AluOpType.max

    with tile.TileContext(nc) as tc:
        with (
            tc.tile_pool(name="consts", bufs=1) as consts,
            tc.tile_pool(name="h0p", bufs=3) as h0p,
            tc.tile_pool(name="h1p", bufs=3) as h1p,
            tc.tile_pool(name="mkp", bufs=4) as mkp,
            tc.tile_pool(name="scp", bufs=6) as scp,
            tc.tile_pool(name="psp", bufs=8, space="PSUM") as psp,
        ):
            w1s = consts.tile([P, 2, 2, CB, P], F8)
            w14s = consts.tile([P, CB, P], F8)
            w2s = consts.tile([P, KW, CB, 2, P], F8)
            wfs = consts.tile([P, CB, P], BF)
            wcs = consts.tile([P, CB, P], BF)
            b1s = consts.tile([P, CB], F32)
            b2s = consts.tile([P, CB], F32)
            invl1s = consts.tile([P, NSLOTS], F32)
            invl2s = consts.tile([P, NSLOTS], F32)
            bfrs = consts.tile([P, P], BF)
            indbs = consts.tile([P, NSLOTS], BF)
            zcol = consts.tile([P, 1], F32)
            out_sb = consts.tile([NSLOTS, P], F32)
            rs1 = consts.tile([P, NSLOTS, CB, NCH], F32)
            rs2 = consts.tile([P, NSLOTS, CB, NCH], F32)
            rs1_red = consts.tile([P, NSLOTS, CB], F32)
            rs2_red = consts.tile([P, NSLOTS, CB], F32)
            rs1b = consts.tile([P, NSLOTS, CB], BF)
            rs2b = consts.tile([P, NSLOTS, CB], BF)

            h0_t = [None] * NSLOTS
            h1_t = [None] * NSLOTS
            mk_t = [None] * NSLOTS
            rest_t = {}

            def emit_load(j, split_first=False):
                L1, L2, c0 = slot_cfg[j]
                if L1 == 0:
                    return
                h0 = h0p.tile([P, 2, H0W], F8, tag="h0")
                h1 = h1p.tile([P, CB, H1W], F8, tag="h1")
                h0_t[j], h1_t[j] = h0, h1
                w = min(L1 + 2, S)
                if split_first:
                    # first two chunks (+halo): lane0 on sync, lane1 on
                    # gpsimd (parallel rings, no ACT-table ahead); the
                    # remainder is loaded later via emit_load_rest so
                    # slot 1's DMAs can issue ahead of it.
                    w0 = min(2 * CH + 6, w)
                    nc.sync.dma_start(h0[:, 0, 2 : 2 + w0], xT[j, :, 0:w0])
                    nc.gpsimd.dma_start(h0[:, 1, 1 : 1 + w0], xT[j, :, 0:w0])
                    rest_t[j] = (w0, w)
                else:
                    nc.sync.dma_start(h0[:, 0, 2 : 2 + w], xT[j, :, 0:w])
                    nc.sync.dma_start(h0[:, 1, 1 : 1 + w], xT[j, :, 0:w])
                nc.vector.memset(h0[:, 0, 0:2], 0.0)
                nc.vector.memset(h0[:, 1, 0:1], 0.0)
                if 2 + w < L1 + 4:
                    nc.vector.memset(h0[:, 0, 2 + w : L1 + 4], 0.0)
                if 1 + w < L1 + 2:
                    nc.vector.memset(h0[:, 1, 1 + w : L1 + 2], 0.0)
                for cb in range(CB):
                    nc.vector.memset(h1[:, cb, 0:2], 0.0)
                    nc.vector.memset(h1[:, cb, 2 + L1 : 4 + L1], 0.0)

            def emit_load_rest(j):
                h0 = h0_t[j]
                if h0 is None or j not in rest_t:
                    return
                a, w = rest_t[j]
                if a < w:
                    nc.sync.dma_start(h0[:, 0, 2 + a : 2 + w], xT[j, :, a:w])
                    nc.sync.dma_start(h0[:, 1, 1 + a : 1 + w], xT[j, :, a:w])

            def emit_mask(j):
                L1, L2, c0 = slot_cfg[j]
                wm = L2 - c0 * CH
                if L1 > 0 and wm > 0:
                    mk = mkp.tile([P, S], BF, tag="mk")
                    mk_t[j] = mk
                    src = msk[j, c0 * CH : c0 * CH + wm]
                    bsrc = bass.AP(
                        tensor=src.tensor, offset=src.offset,
                        ap=[[0, P]] + list(src.ap),
                    )
                    nc.gpsimd.dma_start(mk[:, 0:wm], bsrc)

            def emit_conv1(j):
                L1, L2, c0 = slot_cfg[j]
                h0, h1 = h0_t[j], h1_t[j]
                for c, wc in enumerate(_chunks(L1)):
                    cs = c * CH
                    for cb in range(CB):
                        ps = psp.tile([P, CH], F32, tag="ps")
                        nc.tensor.matmul(
                            ps[:, 0:wc], w1s[:, 0, :, cb, :],
                            h0[:, :, cs : cs + wc],
                            start=True, stop=False, perf_mode=DR,
                        )
                        nc.tensor.matmul(
                            ps[:, 0:wc], w1s[:, 1, :, cb, :],
                            h0[:, :, cs + 2 : cs + 2 + wc],
                            start=False, stop=False, perf_mode=DR,
                        )
                        nc.tensor.matmul(
                            ps[:, 0:wc], w14s[:, cb, :],
                            h0[:, 0, cs + 4 : cs + 4 + wc],
                            start=False, stop=True,
                        )
                        # fused epilogue on DVE: h1 = max(psum + 16*b1, 0)
                        # with rowsum accumulated on every chunk — the fp8
                        # correction only needs a statistical mean of h1, so
                        # the unmasked sum over [0, L1) works (validated);
                        # the host folds 1/(WS*L1) instead of 1/(WS*len).
                        nc.vector.scalar_tensor_tensor(
                            h1[:, cb, 2 + cs : 2 + cs + wc],
                            ps[:, 0:wc], b1s[:, cb : cb + 1],
                            zcol.to_broadcast((P, wc)),
                            ADD, MAX, accum_out=rs1[:, j, cb, c : c + 1],
                        )

            def emit_conv2(j, full_last=False, eng=None):
                L1, L2, c0 = slot_cfg[j]
                h1, mk = h1_t[j], mk_t[j]
                order = list(enumerate(_chunks(L2)))
                if full_last:
                    # tail slot: fully-accumulated chunks first (their
                    # ScalarE accumulator reads overlap the masked chunks),
                    # masked chunks after, SMALLEST last — the kernel tail
                    # then ends in a short ACT + fused DVE mask-accumulate.
                    order = [cw for cw in order if cw[0] < c0] + \
                            sorted([cw for cw in order if cw[0] >= c0],
                                   key=lambda cw: -cw[1])
                for c, wc in order:
                    cs = c * CH
                    for cb in range(CB):
                        ps = psp.tile([P, CH], F32, tag="ps")
                        for k in range(KW):
                            nc.tensor.matmul(
                                ps[:, 0:wc],
                                w2s[:, k, cb, :, :],
                                h1[:, :, cs + k : cs + k + wc],
                                start=(k == 0), stop=(k == KW - 1),
                                perf_mode=DR,
                            )
                        h2 = scp.tile([P, CH], BF, tag="h2")
                        if c < c0:
                            # unmasked for every core in the group:
                            # ReLU + bias + rowsum fused on ScalarE
                            nc.scalar.activation(
                                h2[:, 0:wc], ps[:, 0:wc], RELU,
                                bias=b2s[:, cb : cb + 1],
                                accum_out=rs2[:, j, cb, c : c + 1],
                            )
                        else:
                            nc.scalar.activation(
                                h2[:, 0:wc], ps[:, 0:wc], RELU,
                                bias=b2s[:, cb : cb + 1],
                            )
                            sc = scp.tile([P, CH], BF, tag="sc")
                            eng.scalar_tensor_tensor(
                                sc[:, 0:wc], h2[:, 0:wc], 1.0,
                                mk[:, cs - c0 * CH : cs - c0 * CH + wc],
                                MUL, MUL,
                                accum_out=rs2[:, j, cb, c : c + 1],
                            )

            def _reduce_one(rs, red, rsb, invs, j, nc_):
                if nc_ != 1:
                    for cb in range(CB):
                        if nc_ == 0:
                            nc.vector.memset(red[:, j, cb : cb + 1], 0.0)
                        else:
                            nc.vector.tensor_reduce(
                                red[:, j, cb : cb + 1],
                                rs[:, j, cb, 0:nc_],
                                axis=mybir.AxisListType.X, op=ADD,
                            )
                # fold 1/(scale*len) so the tail is only matvec + bias;
                # bf16 output feeds the single-pass bf16 matvec. A single
                # chunk needs no reduce — multiply straight off the rowsum
                # column (strided AP) to keep the tail chain short.
                src = red[:, j, :] if nc_ != 1 else rs[:, j, :, 0]
                nc.vector.tensor_tensor(
                    rsb[:, j, :], src,
                    invs[:, j : j + 1].to_broadcast((P, CB)), MUL,
                )

            def emit_rs1_reduce(j):
                # rs1 completes with conv1(j)'s last epilogue — reduce early
                L1, L2, c0 = slot_cfg[j]
                _reduce_one(rs1, rs1_red, rs1b, invl1s, j, len(_chunks(L1)))

            def emit_slot_reduce(j):
                L1, L2, c0 = slot_cfg[j]
                _reduce_one(rs2, rs2_red, rs2b, invl2s, j, len(_chunks(L2)))

            # ---- emission order ----
            # PE warmup: the first data DMAs cannot complete before ~12us
            # (engine preamble + descriptor processing); dummy matmuls keep
            # the PE busy so the HAM clock gate is released (8/8 = 2.4 GHz)
            # by the time real matmuls issue.
            warm_w = scp.tile([P, CH], BF, tag="warm")
            warm_ps = psp.tile([P, CH], F32, tag="ps")
            nc.gpsimd.memset(warm_w, 0.0)
            for _ in range(14):
                nc.tensor.matmul(warm_ps, warm_w[:, 0:P], warm_w,
                                 start=True, stop=True)

            # slot 0's first x piece lanes go first on the sync/gpsimd
            # queues; slot 1's load issues before slot 0's remainder;
            # all weights on gpsimd SWDGE; biases early on the scalar
            # queue (the first conv1 epilogue needs b1s at ~14us).
            nc.gpsimd.dma_start(w1s, w1t[:])
            emit_load(0, split_first=True)
            nc.scalar.dma_start(b1s, bias1[:])
            nc.vector.memset(zcol, 0.0)
            emit_load_rest(0)
            emit_load(1)
            nc.gpsimd.dma_start(w14s, w14t[:])
            nc.gpsimd.dma_start(w2s, w2t[:])
            nc.scalar.dma_start(b2s, bias2[:])
            nc.scalar.dma_start(invl1s, invl1[:])
            nc.scalar.dma_start(invl2s, invl2[:])
            emit_mask(0)
            emit_mask(1)
            emit_mask(2)
            emit_mask(3)
            emit_conv1(0)
            emit_rs1_reduce(0)
            emit_load(2)
            emit_conv1(1)
            emit_rs1_reduce(1)
            emit_conv2(0, eng=nc.vector)
            emit_slot_reduce(0)
            emit_load(3)
            emit_conv1(2)
            emit_rs1_reduce(2)
            emit_conv2(1, eng=nc.vector)
            emit_slot_reduce(1)
            nc.scalar.dma_start(wfs, wft[:])
            nc.scalar.dma_start(wcs, wct[:])
            nc.scalar.dma_start(bfrs, bfr[:])
            nc.scalar.dma_start(indbs, indb[:])
            # conv2(2) runs between conv1(3) and conv2(3) so slot 3's h1
            # epilogue (DVE) hides under its matmuls; the kernel tail is
            # then slot 3's single short masked chunk.
            emit_conv1(3)
            emit_rs1_reduce(3)
            emit_conv2(2, eng=nc.vector)
            emit_slot_reduce(2)
            emit_conv2(3, eng=nc.vector)
            emit_slot_reduce(3)
            # single bf16 matvec for all samples, TRANSPOSED: rs is the
            # stationary operand so the result lands as [NSLOTS, P] — the
            # output DMA then needs 4 descriptors instead of 128 (saves
            # ~2us of DMA completion latency on the tail). fp8-correction
            # (rs1, ready early) first, 1x1-conv (rs2, tail-critical) last.
            pooled = psp.tile([P, CH], F32, tag="ps")
            # the final-conv bias rides the matvec as one extra matmul:
            # sum_p (ind_j/128) * bf[d] = ind_j*bf[d], exact in bf16 since
            # 1/128 is a power of two — the output DMA then reads PSUM
            # directly, with no bias-add on the tail chain
            nc.tensor.matmul(
                pooled[0:NSLOTS, 0:P], indbs, bfrs,
                start=True, stop=False,
            )
            ops = [(wcs, rs1b), (wfs, rs2b)]
            i = 0
            for w_, r_ in ops:
                for cb in range(CB):
                    nc.tensor.matmul(
                        pooled[0:NSLOTS, 0:P],
                        r_[:, :, cb],
                        w_[:, cb, :],
                        start=False,
                        stop=(i == 2 * CB - 1),
                    )
                    i += 1
            nc.vector.tensor_copy(out_sb, pooled[0:NSLOTS, 0:P])
            nc.sync.dma_start(out[:], out_sb)

    nc.compile()
    return nc


def _prep(inputs):
    """Host-side: BN folding, fp8 weight packing, length-sorted slots."""
    x = np.ascontiguousarray(np.asarray(inputs["x"], dtype=np.float32))
    spi = np.asarray(inputs["start_padding_indices"]).astype(np.int64).reshape(B)
    W1 = np.asarray(inputs["W1"], np.float32)
    b1 = np.asarray(inputs["b1"], np.float32)
    g1 = np.asarray(inputs["g1"], np.float32)
    be1 = np.asarray(inputs["be1"], np.float32)
    m1 = np.asarray(inputs["m1"], np.float32)
    v1 = np.asarray(inputs["v1"], np.float32)
    W2 = np.asarray(inputs["W2"], np.float32)
    b2 = np.asarray(inputs["b2"], np.float32)
    g2 = np.asarray(inputs["g2"], np.float32)
    be2 = np.asarray(inputs["be2"], np.float32)
    m2 = np.asarray(inputs["m2"], np.float32)
    v2 = np.asarray(inputs["v2"], np.float32)
    Wf = np.asarray(inputs["Wf"], np.float32)
    bf = np.asarray(inputs["bf"], np.float32)

    lens = np.where(spi == -1, S, spi)
    lens = np.clip(lens, 0, S).astype(np.int64)

    order = np.argsort(-lens, kind="stable")
    assign = order.reshape(NSLOTS, NCORES)  # [slot, core] -> sample idx

    slot_cfg = []
    for j in range(NSLOTS):
        lj = lens[assign[j]]
        lmax, lmin = int(lj.max()), int(lj.min())
        if lmax == 0:
            slot_cfg.append((0, 0, 0))
            continue
        L2 = min(math.ceil(lmax / GR) * GR, S)
        L1 = min(math.ceil(min(lmax + 2, S) / GR) * GR, S)
        c0 = min(lmin // CH, len(_chunks(L2)))
        slot_cfg.append((L1, L2, c0))
    slot_cfg = tuple(slot_cfg)

    # fold BN into conv weights/biases
    s1 = g1 / np.sqrt(v1 + EPS)
    W1f = W1 * s1[:, None, None]
    b1f = (b1 - m1) * s1 + be1
    s2 = g2 / np.sqrt(v2 + EPS)
    W2f = W2 * s2[:, None, None]
    b2f = (b2 - m2) * s2 + be2

    # fp8 weights, scaled by WS (scale folded downstream)
    W1q = np.clip(W1f * WS, -240, 240).astype(NP8)   # [C, D, K]
    W2q = np.clip(W2f * WS, -240, 240).astype(NP8)   # [C, C, K]

    # conv1 DoubleRow packs: [d, pair, i, cb, co] for taps 0..3, tap 4 alone
    a1 = np.asarray(W1q).reshape(CB, P, D, KW)                 # [cb, co, d, k]
    w1t = np.ascontiguousarray(
        a1[:, :, :, 0:4].reshape(CB, P, D, 2, 2).transpose(2, 3, 4, 0, 1)
    )                                                          # [d, p, i, cb, co]
    w14t = np.ascontiguousarray(a1[:, :, :, 4].transpose(2, 0, 1))  # [d, cb, co]
    # conv2 DoubleRow pack: pair = input channel block
    a2 = np.asarray(W2q).reshape(CB, P, 2, P, KW)              # [cob, co, i, ci, k]
    w2t = np.ascontiguousarray(a2.transpose(3, 4, 0, 2, 1))    # [ci, k, cob, i, co]

    # fp8 W2 error correction: Wc = 0.5 * Wf @ sum_k(W2f - deq(W2q))
    dW2eff = (W2f - np.asarray(W2q).astype(np.float32) / WS).sum(axis=2)  # [co, ci]
    Wc = 0.5 * (Wf[:, :, 0] @ dW2eff)                          # [d, ci]

    wft = np.ascontiguousarray(
        Wf[:, :, 0].reshape(D, CB, P).transpose(2, 1, 0)
    ).astype(BF16)  # [ci, cb, d]
    wct = np.ascontiguousarray(
        Wc.reshape(D, CB, P).transpose(2, 1, 0)
    ).astype(BF16)
    bias1 = np.ascontiguousarray((WS * b1f).reshape(CB, P).T).astype(np.float32)
    bias2 = np.ascontiguousarray((WS * WS * b2f).reshape(CB, P).T).astype(np.float32)

    bfr_np = np.ascontiguousarray(np.tile(bf[None, :], (P, 1))).astype(BF16)

    t_idx = np.arange(S)
    in_maps = []
    for i in range(NCORES):
        xT_i = np.empty((NSLOTS, P, S), dtype=NP8)
        msk_i = np.zeros((NSLOTS, S), dtype=BF16)
        invl1_i = np.empty((P, NSLOTS), dtype=np.float32)
        invl2_i = np.empty((P, NSLOTS), dtype=np.float32)
        indb_i = np.empty((P, NSLOTS), dtype=BF16)
        for j in range(NSLOTS):
            b_idx = int(assign[j, i])
            L = int(lens[b_idx])
            L1j = slot_cfg[j][0]
            xT_i[j] = np.clip(x[b_idx].T, -240, 240).astype(NP8)
            msk_i[j] = (t_idx < L).astype(BF16)
            # rs1 is the UNMASKED h1 sum over [0, L1): statistical mean for
            # the fp8 correction; zeroed entirely for empty samples
            invl1_i[:, j] = 1.0 / (WS * L1j) if L > 0 else 0.0
            invl2_i[:, j] = 1.0 / (WS * WS * max(L, 1))
            indb_i[:, j] = np.float32((1.0 / P) if L > 0 else 0.0)
        in_maps.append({
            "xT": xT_i, "msk": msk_i,
            "w1t": w1t, "w14t": w14t, "w2t": w2t,
            "wft": wft, "wct": wct,
            "bias1": bias1, "bias2": bias2,
            "invl1": invl1_i, "invl2": invl2_i,
            "bfr": bfr_np, "indb": indb_i,
        })
    return slot_cfg, assign, in_maps


def kernel(**inputs) -> np.ndarray:
    global LAST_RESULTS
    import os

    slot_cfg, assign, in_maps = _prep(inputs)
    nc = _BUILD_CACHE.get(slot_cfg)
    if nc is None:
        nc = _build(slot_cfg)
        _BUILD_CACHE[slot_cfg] = nc

    trace = TRACE or bool(os.environ.get("BASS_TRACE"))
    if trace:
        try:
            import antenv.axon_hooks  # noqa: F401  (absent in some containers)
        except ImportError:
            trace = False
    res = run_bass_kernel_spmd(
        nc, in_maps, core_ids=list(range(NCORES)), trace=trace,
    )
    LAST_RESULTS = res

    pooled = np.zeros((B, D), dtype=np.float32)
    for i in range(NCORES):
        out_i = np.asarray(res.results[i]["out"], dtype=np.float32)  # [NSLOTS, P]
        for j in range(NSLOTS):
            pooled[int(assign[j, i])] = out_i[j, :]
    return pooled
